# revision 35
# baseline (speedup 1.0000x reference)
"""CNF vector-field + exact Jacobian-trace kernel for Trainium2 (8 NeuronCores).

Math: for each sample x (D=32), with inp = [x, t] (33,):
  h1 = tanh(inp @ W1 + b1); h2 = tanh(h1 @ W2 + b2); dx = h2 @ W3 + b3
  div = trace(J),  J = W1r D1 W2 D2 W3  (D_i = diag(1 - h_i^2), W1r = W1[:32])
      = d1^T C d2,  C = W2 * (W3 @ W1r)^T   (elementwise *)
  out = [dx, div]  (B, 33)

Implementation notes:
  - data-parallel over batch (2048 -> 8 x 256), weights replicated
  - feature-major on-device layout: weights are natural pre-transposed lhsT
  - P = -C;  gt = P^T h1sq - (P^T 1);  E = (h2sq - 1) * gt = gt * d2 * (-1)
    div = (-1)^T E  -- the "1 - x^2" affines fold into matmuls / fused DVE ops
  - matmuls run as float32r (TF32-like, 4x faster than fp32 at N>=256)
  - consolidated DMAs via 3-D access patterns; W2 (the big one) issued last
  - engine streams are in-order: emission order is tuned so PE/ACT/DVE/Pool
    overlap (P-chain early, vp after z2, copies on ACT, h2sq on GpSimd)
"""
import sys

for _p in ("/opt/trn_rl_repo", "/root/.axon_site/_ro/trn_rl_repo"):
    if _p not in sys.path:
        sys.path.append(_p)

import numpy as np

B, D, H = 2048, 32, 512
NCORES = 8
BC = B // NCORES          # 256 rows per core
NK = H // 128             # 4 chunks of the hidden dim

_CACHE = {}


def _build(reps=None):
    import contextlib
    import concourse.bass as bass
    import concourse.tile as tile
    from concourse import bacc, mybir
    from concourse.masks import make_identity

    f32 = mybir.dt.float32
    f32r = mybir.dt.float32r
    AF = mybir.ActivationFunctionType
    ALU = mybir.AluOpType

    nc = bacc.Bacc("TRN2", target_bir_lowering=False, debug=False,
                   num_devices=NCORES)

    x_ext = nc.dram_tensor("x", [BC, D + 1], f32, kind="ExternalInput").ap()
    # w1 = [W1; b1] stacked then column-interleaved on host -> (16, 34, 32):
    # w1i[a, r, b] = w1s[r, a*32 + b]. The interleave makes the DMA split
    # into 34*16 non-contiguous descriptors so all 16 HWDGE queues are busy
    # (DMAs that leave queues empty get ~4us-late completion semaphores).
    w1_ext = nc.dram_tensor("w1", [16, D + 2, 32], f32r, kind="ExternalInput").ap()
    w2_ext = nc.dram_tensor("w2", [H, H], f32r, kind="ExternalInput").ap()
    w3_ext = nc.dram_tensor("w3", [H, D], f32r, kind="ExternalInput").ap()
    # colpack cols: 0=+1, 1=-1, 2=u=[0..0,t,1] (34 used), 3:7=b2 column-major
    colp_ext = nc.dram_tensor("colp", [128, 7], f32r, kind="ExternalInput").ap()
    # rowpack: [0:256]=ones, [256:288]=b3; host-padded to (16, 32) rows with
    # data in cols 0:18 so the DMA emits 16 strided descriptors (see w1 note)
    rowp_ext = nc.dram_tensor("rowp", [16, 32], f32r, kind="ExternalInput").ap()
    out_ext = nc.dram_tensor("out", [BC, D + 1], f32, kind="ExternalOutput").ap()

    with tile.TileContext(nc) as tc:
        with tc.tile_pool(name="const", bufs=1) as cpool, \
             tc.tile_pool(name="work", bufs=1) as wpool, \
             tc.tile_pool(name="ps", bufs=1, space="PSUM") as pps, \
             (tc.For_i(0, reps, 1) if reps else contextlib.nullcontext()):

            def big_ps(nm):
                return pps.tile([128, H], f32, name=nm, tag="big", bufs=5)

            def small_ps(nm, shape):
                return pps.tile(shape, f32, name=nm, tag="small", bufs=3)

            # -------- ACT spline-table preload (overlaps the DMA phase) -----
            dm0 = wpool.tile([1, 1], f32, name="dm0")
            dm1 = wpool.tile([1, 1], f32, name="dm1")
            nc.gpsimd.memset(dm0[:, :], 0.0)
            nc.scalar.activation(dm1[:, :], dm0[:, :], AF.Tanh)

            # ------------- input DMAs (few, large; W2 last) -------------
            w1e = cpool.tile([D + 2, H], f32r, name="w1e")   # 0:33 = W1, 33 = b1
            nc.sync.dma_start(
                out=w1e[:, :].rearrange("r (a b) -> r a b", a=16),
                in_=w1_ext.rearrange("a r b -> r a b"))

            colp = cpool.tile([128, 7], f32r, name="colp")
            nc.sync.dma_start(out=colp[:, :], in_=colp_ext[:, :])
            ones_col = colp[:, 0:1]
            neg_col = colp[:, 1:2]
            u = colp[0:D + 2, 2:3]

            w3all = cpool.tile([128, NK * D], f32r, name="w3all")
            nc.sync.dma_start(
                out=w3all[:, :].rearrange("p (k j) -> p k j", k=NK),
                in_=w3_ext.rearrange("(k p) j -> p k j", k=NK))
            w3k = [w3all[:, k * D:(k + 1) * D] for k in range(NK)]

            xall = wpool.tile([128, 2 * (D + 1)], f32, name="xall")
            nc.scalar.dma_start(
                out=xall[:, :].rearrange("p (i c) -> p i c", i=2),
                in_=x_ext.rearrange("(i p) c -> p i c", i=2))

            w2all = cpool.tile([128, NK * H], f32r, name="w2all")
            nc.sync.dma_start(
                out=w2all[:, :].rearrange("p (k j) -> p k j", k=NK),
                in_=w2_ext.rearrange("(k p) j -> p k j", k=NK))
            w2k = [w2all[:, k * H:(k + 1) * H] for k in range(NK)]

            rowp = cpool.tile([1, BC + D], f32r, name="rowp")
            nc.sync.dma_start(
                out=rowp[:, :].rearrange("p (a b) -> p a b", a=16),
                in_=rowp_ext[:, 0:18].rearrange("(o a) b -> o a b", o=1))
            ones_row = rowp[:, 0:BC]
            b3row = rowp[:, BC:BC + D]

            ident = cpool.tile([128, 128], f32, name="ident")
            make_identity(nc, ident[:, :])

            # ------- W3^T (negated): PE transposes + DVE negate-copies -------
            negw3t = wpool.tile([D, H], f32r, name="negw3t")
            for k in range(NK):
                wp = small_ps("w3tp", [D, 128])
                nc.tensor.transpose(wp[:, :], w3k[k].bitcast(f32), ident[:, :])
                nc.vector.tensor_scalar(out=negw3t[:, k * 128:(k + 1) * 128],
                                        in0=wp[:, :], scalar1=-1.0, scalar2=None,
                                        op0=ALU.mult)

            # ---------------- x transpose: A0 = xs^T (32, 256) ----------------
            a0 = wpool.tile([D, BC], f32r, name="a0")
            for i in range(2):
                xp = small_ps("xT", [D + 1, 128])
                nc.tensor.transpose(xp[:, :], xall[:, i * (D + 1):(i + 1) * (D + 1)],
                                    ident[:, :])
                nc.vector.tensor_copy(a0[:, i * 128:(i + 1) * 128], xp[0:D, :])

            # ---- PE HAM warm-up: ~4us of dummy matmuls in the idle entry
            # window so the clock gate is at 2.4GHz when real work starts.
            # They WAW-serialize on one psum slot and are never read.
            for w in range(6):
                warm = pps.tile([128, 128], f32, name="warm", tag="big", bufs=5)
                nc.tensor.matmul(warm[:, :], w3all[:, 0:128], w3all[:, 0:128],
                                 start=True, stop=True)

            # ---------------- layer 1 matmuls, then all tanh ----------------
            z1s = []
            for m in range(NK):
                z1 = big_ps("z1")
                nc.tensor.matmul(z1[:, 0:BC], w1e[0:D, m * 128:(m + 1) * 128],
                                 a0[:, :], start=True, stop=True)
                z1s.append(z1)
            # ---------------- bias1 column = t*W1[32,:] + b1 ----------------
            bias_ps = small_ps("bias_ps", [128, NK])
            for m in range(NK):
                nc.tensor.matmul(bias_ps[:, m:m + 1],
                                 w1e[:, m * 128:(m + 1) * 128].bitcast(f32),
                                 u.bitcast(f32),
                                 start=True, stop=True, skip_group_check=True)
            bias1 = wpool.tile([128, NK], f32, name="bias1")
            nc.scalar.activation(bias1[:, :], bias_ps[:, :], AF.Copy)

            h1t = []
            for m in range(NK):
                h = wpool.tile([128, BC], f32r, name=f"h1t_{m}")
                nc.scalar.activation(h[:, :], z1s[m][:, 0:BC], AF.Tanh,
                                     bias=bias1[:, m:m + 1])
                h1t.append(h)

            # ---------------- P = -(W2 * M^T), M = W3 @ W1r ----------------
            pmat = []
            for m in range(NK):
                mp = big_ps("negMt")
                nc.tensor.matmul(mp[:, :], w1e[0:D, m * 128:(m + 1) * 128],
                                 negw3t[:, :], start=True, stop=True)
                p = cpool.tile([128, H], f32r, name=f"p_{m}")
                nc.vector.tensor_tensor(out=p[:, :], in0=w2k[m].bitcast(f32),
                                        in1=mp[:, :], op=ALU.mult)
                pmat.append(p)

            # ---------------- vP row (early: gates the div tail) ------------
            vp_ps = small_ps("vp_ps", [1, H])
            for k in range(NK):
                nc.tensor.matmul(vp_ps[:, :], ones_col, pmat[k][:, :],
                                 start=(k == 0), stop=(k == NK - 1))
            vneg = wpool.tile([1, H], f32r, name="vneg")
            nc.scalar.activation(vneg[:, :], vp_ps[:, :], AF.Copy, scale=-1.0)

            # ---------------- h1sq on DVE (f32r, feeds gt matmuls) ----------
            h1sq = []
            for m in range(NK):
                sq = wpool.tile([128, BC], f32r, name=f"h1sq_{m}")
                nc.vector.tensor_tensor(out=sq[:, :], in0=h1t[m][:, :].bitcast(f32),
                                        in1=h1t[m][:, :].bitcast(f32), op=ALU.mult)
                h1sq.append(sq)

            # ---------------- layer 2 ----------------
            h2t = []
            for m in range(NK):
                z2 = big_ps("z2")
                for k in range(NK):
                    nc.tensor.matmul(z2[:, 0:BC],
                                     w2k[k][:, m * 128:(m + 1) * 128],
                                     h1t[k][:, :],
                                     start=(k == 0), stop=(k == NK - 1))
                h = wpool.tile([128, BC], f32r, name=f"h2t_{m}")
                nc.scalar.activation(h[:, :], z2[:, 0:BC], AF.Tanh,
                                     bias=colp[:, 3 + m:4 + m].bitcast(f32))
                h2t.append(h)

            # ---------------- h2sq on GpSimd (SBUF only) ----------------
            h2sq = []
            for m in range(NK):
                sq = wpool.tile([128, BC], f32, name=f"h2sq_{m}")
                nc.gpsimd.tensor_tensor(out=sq[:, :], in0=h2t[m][:, :].bitcast(f32),
                                        in1=h2t[m][:, :].bitcast(f32), op=ALU.mult)
                h2sq.append(sq)

            # ------- gt = P^T h1sq - vP ; E = (h2sq - 1) * gt = -gt*d2 -------
            ee = []
            for m in range(NK):
                gt = big_ps("gt")
                for k in range(NK):
                    nc.tensor.matmul(gt[:, 0:BC],
                                     pmat[k][:, m * 128:(m + 1) * 128],
                                     h1sq[k][:, :],
                                     start=(k == 0), stop=False)
                nc.tensor.matmul(gt[:, 0:BC], vneg[:, m * 128:(m + 1) * 128],
                                 ones_row, start=False, stop=True)
                e = wpool.tile([128, BC], f32r, name=f"e_{m}")
                nc.vector.scalar_tensor_tensor(out=e[:, :], in0=h2sq[m][:, :],
                                               scalar=1.0, in1=gt[:, 0:BC],
                                               op0=ALU.subtract, op1=ALU.mult)
                ee.append(e)

            # -------- dx = W3^T h2 + b3 ; div = (-1)^T E --------
            dx_ps = small_ps("dx_ps", [D, BC])
            for k in range(NK):
                nc.tensor.matmul(dx_ps[:, :], w3k[k], h2t[k][:, :],
                                 start=(k == 0), stop=False)
            nc.tensor.matmul(dx_ps[:, :], b3row, ones_row,
                             start=False, stop=True)
            outt = wpool.tile([D + 1, BC], f32, name="outt")
            nc.scalar.activation(outt[0:D, :], dx_ps[:, :], AF.Copy)
            div_ps = small_ps("div_ps", [1, BC])
            for k in range(NK):
                nc.tensor.matmul(div_ps[:, :], neg_col, ee[k][:, :],
                                 start=(k == 0), stop=(k == NK - 1))
            nc.scalar.activation(outt[D:D + 1, :], div_ps[:, :], AF.Copy)

            # ------- transpose back to (256, 33) and store -------
            outs = wpool.tile([128, 2 * (D + 1)], f32, name="outs")
            for i in range(2):
                op = small_ps("outP", [128, D + 1])
                nc.tensor.transpose(op[:, :], outt[:, i * 128:(i + 1) * 128],
                                    ident[0:D + 1, 0:D + 1])
                nc.scalar.activation(outs[:, i * (D + 1):(i + 1) * (D + 1)],
                                     op[:, :], AF.Copy)
            nc.sync.dma_start(
                out=out_ext.rearrange("(i p) c -> p i c", i=2),
                in_=outs[:, :].rearrange("p (i c) -> p i c", i=2))

    nc.compile()
    return nc


def _get_nc():
    if "nc" not in _CACHE:
        _CACHE["nc"] = _build()
    return _CACHE["nc"]


def _prep_inputs(t, x, W1, b1, W2, b2, W3, b3):
    t = np.asarray(t, dtype=np.float32)
    x = np.ascontiguousarray(np.asarray(x, dtype=np.float32))
    W1 = np.asarray(W1, dtype=np.float32)
    b1 = np.asarray(b1, dtype=np.float32)
    w1s = np.concatenate([W1, b1.reshape(1, H)], axis=0)
    w1s = np.ascontiguousarray(
        w1s.reshape(D + 2, 16, 32).transpose(1, 0, 2))  # (16, 34, 32)
    W2 = np.ascontiguousarray(np.asarray(W2, dtype=np.float32))
    W3 = np.ascontiguousarray(np.asarray(W3, dtype=np.float32))
    colp = np.zeros((128, 7), dtype=np.float32)
    colp[:, 0] = 1.0
    colp[:, 1] = -1.0
    colp[D, 2] = t.ravel()[0]
    colp[D + 1, 2] = 1.0
    colp[:, 3:7] = np.asarray(b2, dtype=np.float32).reshape(NK, 128).T
    rowv = np.ones(BC + D, dtype=np.float32)
    rowv[BC:] = np.asarray(b3, dtype=np.float32)
    rowp = np.zeros((16, 32), dtype=np.float32)
    rowp[:, 0:18] = rowv.reshape(16, 18)
    return x, w1s, W2, W3, colp, rowp


def kernel(t, x, W1, b1, W2, b2, W3, b3):
    from concourse.bass_utils import run_bass_kernel_spmd

    nc = _get_nc()
    x, w1s, W2, W3, colp, rowp = _prep_inputs(t, x, W1, b1, W2, b2, W3, b3)
    in_maps = []
    for i in range(NCORES):
        in_maps.append({
            "x": np.ascontiguousarray(x[i * BC:(i + 1) * BC]),
            "w1": w1s, "w2": W2, "w3": W3,
            "colp": colp, "rowp": rowp,
        })
    res = run_bass_kernel_spmd(nc, in_maps, core_ids=list(range(NCORES)))
    return np.concatenate([res.results[i]["out"] for i in range(NCORES)], axis=0)


# revision 36
# speedup vs baseline: 1.0679x; 1.0679x over previous
"""CNF vector-field + exact Jacobian-trace kernel for Trainium2 (8 NeuronCores).

Math: for each sample x (D=32), with inp = [x, t] (33,):
  h1 = tanh(inp @ W1 + b1); h2 = tanh(h1 @ W2 + b2); dx = h2 @ W3 + b3
  div = trace(J),  J = W1r D1 W2 D2 W3  (D_i = diag(1 - h_i^2), W1r = W1[:32])
      = d1^T C d2,  C = W2 * (W3 @ W1r)^T   (elementwise *)
  out = [dx, div]  (B, 33)

Implementation notes:
  - data-parallel over batch (2048 -> 8 x 256), weights replicated
  - feature-major on-device layout: weights are natural pre-transposed lhsT
  - P = -C;  gt = P^T h1sq - (P^T 1);  E = (h2sq - 1) * gt = gt * d2 * (-1)
    div = (-1)^T E  -- the "1 - x^2" affines fold into matmuls / fused DVE ops
  - matmuls run as float32r (TF32-like, 4x faster than fp32 at N>=256)
  - consolidated DMAs via 3-D access patterns; W2 (the big one) issued last
  - engine streams are in-order: emission order is tuned so PE/ACT/DVE/Pool
    overlap (P-chain early, vp after z2, copies on ACT, h2sq on GpSimd)
"""
import sys

for _p in ("/opt/trn_rl_repo", "/root/.axon_site/_ro/trn_rl_repo"):
    if _p not in sys.path:
        sys.path.append(_p)

import numpy as np

B, D, H = 2048, 32, 512
NCORES = 8
BC = B // NCORES          # 256 rows per core
NK = H // 128             # 4 chunks of the hidden dim

_CACHE = {}


def _build(reps=None):
    import contextlib
    import concourse.bass as bass
    import concourse.tile as tile
    from concourse import bacc, mybir
    from concourse.masks import make_identity

    f32 = mybir.dt.float32
    f32r = mybir.dt.float32r
    AF = mybir.ActivationFunctionType
    ALU = mybir.AluOpType

    nc = bacc.Bacc("TRN2", target_bir_lowering=False, debug=False,
                   num_devices=NCORES)

    x_ext = nc.dram_tensor("x", [BC, D + 1], f32, kind="ExternalInput").ap()
    # w1 = [W1; b1] stacked then column-interleaved on host -> (16, 34, 32):
    # w1i[a, r, b] = w1s[r, a*32 + b]. The interleave makes the DMA split
    # into 34*16 non-contiguous descriptors so all 16 HWDGE queues are busy
    # (DMAs that leave queues empty get ~4us-late completion semaphores).
    w1_ext = nc.dram_tensor("w1", [16, D + 2, 32], f32r, kind="ExternalInput").ap()
    w2_ext = nc.dram_tensor("w2", [H, H], f32r, kind="ExternalInput").ap()
    w3_ext = nc.dram_tensor("w3", [H, D], f32r, kind="ExternalInput").ap()
    # colpack cols: 0=+1, 1=-1, 2=u=[0..0,t,1] (34 used), 3:7=b2 column-major
    colp_ext = nc.dram_tensor("colp", [128, 7], f32r, kind="ExternalInput").ap()
    # rowpack: [0:256]=ones, [256:288]=b3; host-padded to (16, 32) rows with
    # data in cols 0:18 so the DMA emits 16 strided descriptors (see w1 note)
    rowp_ext = nc.dram_tensor("rowp", [16, 32], f32r, kind="ExternalInput").ap()
    out_ext = nc.dram_tensor("out", [BC, D + 1], f32, kind="ExternalOutput").ap()

    with tile.TileContext(nc) as tc:
        with tc.tile_pool(name="const", bufs=1) as cpool, \
             tc.tile_pool(name="work", bufs=1) as wpool, \
             tc.tile_pool(name="ps", bufs=1, space="PSUM") as pps, \
             (tc.For_i(0, reps, 1) if reps else contextlib.nullcontext()):

            def big_ps(nm):
                return pps.tile([128, H], f32, name=nm, tag="big", bufs=5)

            def small_ps(nm, shape):
                return pps.tile(shape, f32, name=nm, tag="small", bufs=3)

            # -------- ACT spline-table preload (overlaps the DMA phase) -----
            dm0 = wpool.tile([1, 1], f32, name="dm0")
            dm1 = wpool.tile([1, 1], f32, name="dm1")
            nc.gpsimd.memset(dm0[:, :], 0.0)
            nc.scalar.activation(dm1[:, :], dm0[:, :], AF.Tanh)

            # ------------- input DMAs (few, large; W2 last) -------------
            w1e = cpool.tile([D + 2, H], f32r, name="w1e")   # 0:33 = W1, 33 = b1
            nc.sync.dma_start(
                out=w1e[:, :].rearrange("r (a b) -> r a b", a=16),
                in_=w1_ext.rearrange("a r b -> r a b"))

            colp = cpool.tile([128, 7], f32r, name="colp")
            nc.sync.dma_start(out=colp[:, :], in_=colp_ext[:, :])
            ones_col = colp[:, 0:1]
            neg_col = colp[:, 1:2]
            u = colp[0:D + 2, 2:3]

            w3all = cpool.tile([128, NK * D], f32r, name="w3all")
            nc.sync.dma_start(
                out=w3all[:, :].rearrange("p (k j) -> p k j", k=NK),
                in_=w3_ext.rearrange("(k p) j -> p k j", k=NK))
            w3k = [w3all[:, k * D:(k + 1) * D] for k in range(NK)]

            xall = wpool.tile([128, 2 * (D + 1)], f32, name="xall")
            nc.scalar.dma_start(
                out=xall[:, :].rearrange("p (i c) -> p i c", i=2),
                in_=x_ext.rearrange("(i p) c -> p i c", i=2))

            w2all = cpool.tile([128, NK * H], f32r, name="w2all")
            nc.sync.dma_start(
                out=w2all[:, :].rearrange("p (k j) -> p k j", k=NK),
                in_=w2_ext.rearrange("(k p) j -> p k j", k=NK))
            w2k = [w2all[:, k * H:(k + 1) * H] for k in range(NK)]

            rowp = cpool.tile([1, BC + D], f32r, name="rowp")
            nc.sync.dma_start(
                out=rowp[:, :].rearrange("p (a b) -> p a b", a=16),
                in_=rowp_ext[:, 0:18].rearrange("(o a) b -> o a b", o=1))
            ones_row = rowp[:, 0:BC]
            b3row = rowp[:, BC:BC + D]

            ident = cpool.tile([128, 128], f32, name="ident")
            make_identity(nc, ident[:, :])

            # ------- W3^T (negated): PE transposes + DVE negate-copies -------
            negw3t = wpool.tile([D, H], f32r, name="negw3t")
            for k in range(NK):
                wp = small_ps("w3tp", [D, 128])
                nc.tensor.transpose(wp[:, :], w3k[k].bitcast(f32), ident[:, :])
                nc.vector.tensor_scalar(out=negw3t[:, k * 128:(k + 1) * 128],
                                        in0=wp[:, :], scalar1=-1.0, scalar2=None,
                                        op0=ALU.mult)

            # ---------------- x transpose: A0 = xs^T (32, 256) ----------------
            a0 = wpool.tile([D, BC], f32r, name="a0")
            for i in range(2):
                xp = small_ps("xT", [D + 1, 128])
                nc.tensor.transpose(xp[:, :], xall[:, i * (D + 1):(i + 1) * (D + 1)],
                                    ident[:, :])
                nc.vector.tensor_copy(a0[:, i * 128:(i + 1) * 128], xp[0:D, :])

            # ---------------- layer 1 matmuls, then all tanh ----------------
            z1s = []
            for m in range(NK):
                z1 = big_ps("z1")
                nc.tensor.matmul(z1[:, 0:BC], w1e[0:D, m * 128:(m + 1) * 128],
                                 a0[:, :], start=True, stop=True)
                z1s.append(z1)
            # ---------------- bias1 column = t*W1[32,:] + b1 ----------------
            bias_ps = small_ps("bias_ps", [128, NK])
            for m in range(NK):
                nc.tensor.matmul(bias_ps[:, m:m + 1],
                                 w1e[:, m * 128:(m + 1) * 128].bitcast(f32),
                                 u.bitcast(f32),
                                 start=True, stop=True, skip_group_check=True)
            bias1 = wpool.tile([128, NK], f32, name="bias1")
            nc.scalar.activation(bias1[:, :], bias_ps[:, :], AF.Copy)

            h1t = []
            for m in range(NK):
                h = wpool.tile([128, BC], f32r, name=f"h1t_{m}")
                nc.scalar.activation(h[:, :], z1s[m][:, 0:BC], AF.Tanh,
                                     bias=bias1[:, m:m + 1])
                h1t.append(h)

            # ---------------- P = -(W2 * M^T), M = W3 @ W1r ----------------
            pmat = []
            for m in range(NK):
                mp = big_ps("negMt")
                nc.tensor.matmul(mp[:, :], w1e[0:D, m * 128:(m + 1) * 128],
                                 negw3t[:, :], start=True, stop=True)
                p = cpool.tile([128, H], f32r, name=f"p_{m}")
                nc.vector.tensor_tensor(out=p[:, :], in0=w2k[m].bitcast(f32),
                                        in1=mp[:, :], op=ALU.mult)
                pmat.append(p)

            # ---------------- vP row (early: gates the div tail) ------------
            vp_ps = small_ps("vp_ps", [1, H])
            for k in range(NK):
                nc.tensor.matmul(vp_ps[:, :], ones_col, pmat[k][:, :],
                                 start=(k == 0), stop=(k == NK - 1))
            vneg = wpool.tile([1, H], f32r, name="vneg")
            nc.scalar.activation(vneg[:, :], vp_ps[:, :], AF.Copy, scale=-1.0)

            # ---------------- h1sq on DVE (f32r, feeds gt matmuls) ----------
            h1sq = []
            for m in range(NK):
                sq = wpool.tile([128, BC], f32r, name=f"h1sq_{m}")
                nc.vector.tensor_tensor(out=sq[:, :], in0=h1t[m][:, :].bitcast(f32),
                                        in1=h1t[m][:, :].bitcast(f32), op=ALU.mult)
                h1sq.append(sq)

            # ---------------- layer 2 ----------------
            h2t = []
            for m in range(NK):
                z2 = big_ps("z2")
                for k in range(NK):
                    nc.tensor.matmul(z2[:, 0:BC],
                                     w2k[k][:, m * 128:(m + 1) * 128],
                                     h1t[k][:, :],
                                     start=(k == 0), stop=(k == NK - 1))
                h = wpool.tile([128, BC], f32r, name=f"h2t_{m}")
                nc.scalar.activation(h[:, :], z2[:, 0:BC], AF.Tanh,
                                     bias=colp[:, 3 + m:4 + m].bitcast(f32))
                h2t.append(h)

            # ---------------- h2sq on GpSimd (SBUF only) ----------------
            h2sq = []
            for m in range(NK):
                sq = wpool.tile([128, BC], f32, name=f"h2sq_{m}")
                nc.gpsimd.tensor_tensor(out=sq[:, :], in0=h2t[m][:, :].bitcast(f32),
                                        in1=h2t[m][:, :].bitcast(f32), op=ALU.mult)
                h2sq.append(sq)

            # ------- gt = P^T h1sq - vP ; E = (h2sq - 1) * gt = -gt*d2 -------
            ee = []
            for m in range(NK):
                gt = big_ps("gt")
                for k in range(NK):
                    nc.tensor.matmul(gt[:, 0:BC],
                                     pmat[k][:, m * 128:(m + 1) * 128],
                                     h1sq[k][:, :],
                                     start=(k == 0), stop=False)
                nc.tensor.matmul(gt[:, 0:BC], vneg[:, m * 128:(m + 1) * 128],
                                 ones_row, start=False, stop=True)
                e = wpool.tile([128, BC], f32r, name=f"e_{m}")
                nc.vector.scalar_tensor_tensor(out=e[:, :], in0=h2sq[m][:, :],
                                               scalar=1.0, in1=gt[:, 0:BC],
                                               op0=ALU.subtract, op1=ALU.mult)
                ee.append(e)

            # -------- dx = W3^T h2 + b3 ; div = (-1)^T E --------
            dx_ps = small_ps("dx_ps", [D, BC])
            for k in range(NK):
                nc.tensor.matmul(dx_ps[:, :], w3k[k], h2t[k][:, :],
                                 start=(k == 0), stop=False)
            nc.tensor.matmul(dx_ps[:, :], b3row, ones_row,
                             start=False, stop=True)
            outt = wpool.tile([D + 1, BC], f32, name="outt")
            nc.scalar.activation(outt[0:D, :], dx_ps[:, :], AF.Copy)
            div_ps = small_ps("div_ps", [1, BC])
            for k in range(NK):
                nc.tensor.matmul(div_ps[:, :], neg_col, ee[k][:, :],
                                 start=(k == 0), stop=(k == NK - 1))
            nc.scalar.activation(outt[D:D + 1, :], div_ps[:, :], AF.Copy)

            # ------- transpose back to (256, 33) and store -------
            outs = wpool.tile([128, 2 * (D + 1)], f32, name="outs")
            for i in range(2):
                op = small_ps("outP", [128, D + 1])
                nc.tensor.transpose(op[:, :], outt[:, i * 128:(i + 1) * 128],
                                    ident[0:D + 1, 0:D + 1])
                nc.scalar.activation(outs[:, i * (D + 1):(i + 1) * (D + 1)],
                                     op[:, :], AF.Copy)
            nc.sync.dma_start(
                out=out_ext.rearrange("(i p) c -> p i c", i=2),
                in_=outs[:, :].rearrange("p (i c) -> p i c", i=2))

    nc.compile()
    return nc


def _get_nc():
    if "nc" not in _CACHE:
        _CACHE["nc"] = _build()
    return _CACHE["nc"]


def _prep_inputs(t, x, W1, b1, W2, b2, W3, b3):
    t = np.asarray(t, dtype=np.float32)
    x = np.ascontiguousarray(np.asarray(x, dtype=np.float32))
    W1 = np.asarray(W1, dtype=np.float32)
    b1 = np.asarray(b1, dtype=np.float32)
    w1s = np.concatenate([W1, b1.reshape(1, H)], axis=0)
    w1s = np.ascontiguousarray(
        w1s.reshape(D + 2, 16, 32).transpose(1, 0, 2))  # (16, 34, 32)
    W2 = np.ascontiguousarray(np.asarray(W2, dtype=np.float32))
    W3 = np.ascontiguousarray(np.asarray(W3, dtype=np.float32))
    colp = np.zeros((128, 7), dtype=np.float32)
    colp[:, 0] = 1.0
    colp[:, 1] = -1.0
    colp[D, 2] = t.ravel()[0]
    colp[D + 1, 2] = 1.0
    colp[:, 3:7] = np.asarray(b2, dtype=np.float32).reshape(NK, 128).T
    rowv = np.ones(BC + D, dtype=np.float32)
    rowv[BC:] = np.asarray(b3, dtype=np.float32)
    rowp = np.zeros((16, 32), dtype=np.float32)
    rowp[:, 0:18] = rowv.reshape(16, 18)
    return x, w1s, W2, W3, colp, rowp


def kernel(t, x, W1, b1, W2, b2, W3, b3):
    from concourse.bass_utils import run_bass_kernel_spmd

    nc = _get_nc()
    x, w1s, W2, W3, colp, rowp = _prep_inputs(t, x, W1, b1, W2, b2, W3, b3)
    in_maps = []
    for i in range(NCORES):
        in_maps.append({
            "x": np.ascontiguousarray(x[i * BC:(i + 1) * BC]),
            "w1": w1s, "w2": W2, "w3": W3,
            "colp": colp, "rowp": rowp,
        })
    res = run_bass_kernel_spmd(nc, in_maps, core_ids=list(range(NCORES)))
    return np.concatenate([res.results[i]["out"] for i in range(NCORES)], axis=0)
